# revision 41
# baseline (speedup 1.0000x reference)
"""Trainium2 Bass kernel for AnemllQATLinear (fake-quant linear + LoRA + bias).

Math (per reference):
    scales = clip(scale_A @ scale_B, 1e-8)              # [OUT, IN], rank-4
    n      = w / scales
    q      = clip(round((n + 1) / step), 0, 15)         # step = 2/15
    w_q    = lut[q] * scales                            # lut affine: lut[q] = a + b*q
    y      = x @ w_q.T + bias + 2.0 * (x @ lora_A.T) @ lora_B.T

Strategy (8 NeuronCores, 4 row-groups x 2 col-groups):
    Each core gets x rows R=2048 and weight rows (out features) O=2048.
    - Fake-quant computed on-chip arithmetically (affine LUT; round via
      the +/-1.5*2^23 magic trick, which is round-half-even like jnp.round).
    - Quantized weight W_eff converted to bf16, transposed via the DMA
      x-bar into [in, out] layout (DRAM bounce), streamed per o-chunk.
    - x cast f32->bf16 during SWDGE DMA, x-bar transposed to xT resident.
    - Main matmul in bf16: psum[r, o] += xT.T @ weffT, with the LoRA
      term (rank-16) and bias (rank-1) accumulated into the same PSUM
      group as extra matmuls.
    - Output written f32, assembled (concat) on host.
"""

import os
import numpy as np

import concourse.bass as bass
import concourse.tile as tile
from concourse import bacc, mybir

F32 = mybir.dt.float32
F32R = mybir.dt.float32r
BF16 = mybir.dt.bfloat16
MAGIC = 12582912.0  # 1.5 * 2**23
LUT_SIZE = 16
STEP_INV = (LUT_SIZE - 1) / 2.0  # 7.5

# full problem shapes
B_FULL, S_FULL, IN_FULL, OUT_FULL = 4, 2048, 4096, 4096
RANK, LORA_R = 4, 16
R_GROUPS, O_GROUPS = 4, 2
N_CORES = 8


def build_nc(R, O, I, lut_a, lut_b, OC=256, IC=512, nonaffine_lut=None):
    """Build the single-core graph (SPMD-launched on all 8 cores).

    R: x rows per core; O: out features per core; I: contraction dim.
    OC: o-chunk for the main matmul (moving free dim). IC: i-chunk for quant.
    """
    KT = I // 128          # number of 128-wide i (contraction) tiles
    RT = R // 128          # r tiles
    ZC = min(512, R)       # z (lora) accumulation chunk of rows
    assert O % OC == 0 and OC % 128 == 0 and I % IC == 0 and IC % 128 == 0

    nc = bacc.Bacc(None, target_bir_lowering=False, debug=False)

    x_in = nc.declare_dram_parameter("x", [R, I], F32, isOutput=False)
    w_in = nc.declare_dram_parameter("w", [O, I], F32, isOutput=False)
    sAT_in = nc.declare_dram_parameter("sAT", [RANK, O], F32, isOutput=False)
    sB_in = nc.declare_dram_parameter("sB", [RANK, I], F32, isOutput=False)
    bias_in = nc.declare_dram_parameter("bias", [1, O], F32, isOutput=False)
    lAT_in = nc.declare_dram_parameter("lAT", [I, LORA_R], F32, isOutput=False)
    lBT_in = nc.declare_dram_parameter("lBT", [LORA_R, O], F32, isOutput=False)
    out_ext = nc.declare_dram_parameter("out", [O, R], F32, isOutput=True)

    with tile.TileContext(nc) as tc:
        with              tc.tile_pool(name="const", bufs=1) as const_pool, \
             tc.tile_pool(name="xside", bufs=3) as x_pool, \
             tc.tile_pool(name="xT", bufs=1) as xT_pool, \
             tc.tile_pool(name="wload", bufs=2) as w_pool, \
             tc.tile_pool(name="qs", bufs=2) as s_pool, \
             tc.tile_pool(name="qchain", bufs=2) as chain_pool, \
             tc.tile_pool(name="qout", bufs=2) as wq_pool, \
             tc.tile_pool(name="weffc", bufs=3) as weff_pool, \
             tc.tile_pool(name="ysb", bufs=2) as y_pool, \
             tc.tile_pool(name="smallio", bufs=2) as small_pool, \
             tc.tile_pool(name="ps_s", bufs=2, space="PSUM") as psum_s, \
             tc.tile_pool(name="ps_y", bufs=4, space="PSUM") as psum_y, \
             tc.tile_pool(name="ps_z", bufs=2, space="PSUM") as psum_z:

            # ---- constants / small inputs ----
            lAT_sb = const_pool.tile([128, KT, LORA_R], BF16)
            nc.gpsimd.dma_start(
                out=lAT_sb[:],
                in_=lAT_in.rearrange("(kt p) r -> p kt r", p=128),
            )
            lBT2_sb = const_pool.tile([LORA_R, O], BF16)
            nc.gpsimd.dma_start(out=lBT2_sb[:], in_=lBT_in[:, :])  # cast f32->bf16
            bias_cols = const_pool.tile([128, O // 128], F32)
            nc.gpsimd.dma_start(
                out=bias_cols[:],
                in_=bias_in.rearrange("1 (ot p) -> p ot", p=128))
            z_sb = const_pool.tile([LORA_R, R], BF16)

            # ---- x side: cast to bf16, x-bar transpose into per-chunk xT
            # tiles (finer tiles -> finer scheduler dependencies), with the
            # LoRA z matmuls interleaved per chunk.
            RC = min(512, R)
            NCH = R // RC
            xT_chunks = [xT_pool.tile([128, KT, RC], BF16, name=f"xT_{j}")
                         for j in range(NCH)]
            XH = 2  # halve transpose count; per-instr overhead dominates

            def x_side():
                for j in range(NCH):
                    for h in range(XH):
                        for rt in range(j * (RC // 128), (j + 1) * (RC // 128)):
                            xbf = x_pool.tile([128, I // XH], BF16, tag="xbf")
                            nc.gpsimd.dma_start(
                                out=xbf[:],
                                in_=x_in[rt * 128:(rt + 1) * 128,
                                         h * (I // XH):(h + 1) * (I // XH)],
                            )
                            lo = (rt % (RC // 128)) * 128
                            nc.sync.dma_start(
                                out=xT_chunks[j][:,
                                                 h * (KT // XH):(h + 1) * (KT // XH),
                                                 lo:lo + 128],
                                in_=xbf[:],
                                transpose=True,
                            )

            def z_chunk(j):
                zp = psum_z.tile([LORA_R, RC], F32, space="PSUM")
                for kt in range(KT):
                    nc.tensor.matmul(
                        zp[:],
                        lAT_sb[:, kt, :],
                        xT_chunks[j][:, kt, :],
                        start=(kt == 0), stop=(kt == KT - 1),
                    )
                # fold the *2.0 LoRA scaling into the psum evacuation
                nc.vector.tensor_scalar(z_sb[:, j * RC:(j + 1) * RC], zp[:],
                                        2.0, None, op0=mybir.AluOpType.mult)

            # ---- main loop: software-pipelined by one o-tile: quant(ot+1) is
            # traced BEFORE the matmuls of ot so every engine queue (PE's
            # scales-mm especially) runs the next tile's quant work while the
            # current column's matmuls stream.

            def quant(ot):
                """Quantize weight o-tile `ot` into a fresh weffT tile."""
                weffT_c = weff_pool.tile([128, KT, 128], BF16, tag="weff",
                                         name=f"weff_{ot}")
                sAT_f = small_pool.tile([RANK, 128], F32, tag="sATf")
                nc.scalar.dma_start(
                    out=sAT_f[:], in_=sAT_in[:, ot * 128:(ot + 1) * 128])
                sAT_t = small_pool.tile([RANK, 128], F32R, tag="sAT")
                nc.vector.tensor_copy(sAT_t[:], sAT_f[:])
                for ic in range(I // IC):
                    w_t = w_pool.tile([128, IC], F32, tag="w")
                    nc.scalar.dma_start(
                        out=w_t[:],
                        in_=w_in[ot * 128:(ot + 1) * 128, ic * IC:(ic + 1) * IC],
                    )
                    sB_f = small_pool.tile([RANK, IC], F32, tag="sBf")
                    nc.scalar.dma_start(
                        out=sB_f[:], in_=sB_in[:, ic * IC:(ic + 1) * IC])
                    sB_t = small_pool.tile([RANK, IC], F32R, tag="sBr")
                    nc.vector.tensor_copy(sB_t[:], sB_f[:])
                    sp = psum_s.tile([128, IC], F32, space="PSUM")
                    nc.tensor.matmul(sp[:], sAT_t[:], sB_t[:],
                                     start=True, stop=True)
                    # s' = max(sp, eps) * step  (so 1/s' = 7.5/s, and
                    # s' is also the final rescale factor: wq=(q+a/b)*s')
                    s_t = s_pool.tile([128, IC], F32, tag="s")
                    nc.vector.tensor_scalar(s_t[:], sp[:], 1e-8, 2.0 / 15.0,
                                            op0=mybir.AluOpType.max,
                                            op1=mybir.AluOpType.mult)
                    r_t = chain_pool.tile([128, IC], F32, tag="chain")
                    nc.vector.reciprocal_approx_fast(r_t[:], s_t[:])
                    n_t = chain_pool.tile([128, IC], F32, tag="chain")
                    nc.vector.tensor_tensor(n_t[:], w_t[:], r_t[:],
                                            op=mybir.AluOpType.mult)
                    t_t = chain_pool.tile([128, IC], F32, tag="chain")
                    nc.vector.tensor_scalar(t_t[:], n_t[:], STEP_INV, MAGIC,
                                            op0=mybir.AluOpType.add,
                                            op1=mybir.AluOpType.add)
                    q_t = chain_pool.tile([128, IC], F32, tag="chain")
                    nc.vector.tensor_scalar(q_t[:], t_t[:], MAGIC,
                                            float(LUT_SIZE - 1),
                                            op0=mybir.AluOpType.subtract,
                                            op1=mybir.AluOpType.min)
                    u_t = chain_pool.tile([128, IC], F32, tag="chain")
                    if nonaffine_lut is None:
                        nc.vector.tensor_scalar(u_t[:], q_t[:], 0.0,
                                                float(lut_a / lut_b),
                                                op0=mybir.AluOpType.max,
                                                op1=mybir.AluOpType.add)
                    else:
                        # generic LUT: u = (lut[0] + sum_k d_k*(q >= k-0.5))/b
                        lut = nonaffine_lut
                        b_scl = 2.0 / 15.0
                        acc = chain_pool.tile([128, IC], F32, tag="nacc")
                        nc.vector.tensor_scalar(acc[:], q_t[:], 0.0,
                                                float(lut[0]) / b_scl,
                                                op0=mybir.AluOpType.mult,
                                                op1=mybir.AluOpType.add)
                        for k in range(1, LUT_SIZE):
                            d_k = float(lut[k] - lut[k - 1])
                            ind = chain_pool.tile([128, IC], F32, tag="nind")
                            nc.vector.tensor_scalar(ind[:], q_t[:], k - 0.5,
                                                    d_k / b_scl,
                                                    op0=mybir.AluOpType.is_ge,
                                                    op1=mybir.AluOpType.mult)
                            acc2 = chain_pool.tile([128, IC], F32, tag="nacc")
                            nc.vector.tensor_tensor(acc2[:], acc[:], ind[:],
                                                    op=mybir.AluOpType.add)
                            acc = acc2
                        u_t = acc
                    if nonaffine_lut is not None or abs(lut_b - 2.0 / 15.0) < 1e-12:
                        s_fin = s_t
                    else:
                        s_fin = s_pool.tile([128, IC], F32, tag="s2")
                        nc.vector.tensor_scalar(s_fin[:], s_t[:],
                                                float(lut_b / (2.0 / 15.0)),
                                                None, op0=mybir.AluOpType.mult)
                    # batch four ic-chunks into one wq tile -> one transpose
                    if ic % 4 == 0:
                        wq_t = wq_pool.tile([128, 4 * IC], BF16, tag="wq")
                    part = (ic % 4) * IC
                    nc.vector.tensor_tensor(wq_t[:, part:part + IC], u_t[:],
                                            s_fin[:], op=mybir.AluOpType.mult)
                    if ic % 4 == 3:
                        nc.sync.dma_start(
                            out=weffT_c[:, (ic - 3) * (IC // 128):
                                        (ic + 1) * (IC // 128), :],
                            in_=wq_t[:],
                            transpose=True,
                        )
                return weffT_c

            def column_chunk(ot, weffT_c, j, with_z=False):
                """One r-chunk of yT[o_tile] = W_eff[o_tile] @ x.T + lora;
                bias folds into the ACT psum evacuation."""
                yp = psum_y.tile([128, RC], F32, space="PSUM",
                                 tag="yp", name=f"yp{j}_{ot}")
                for kt in range(KT):
                    nc.tensor.matmul(
                        yp[:],
                        weffT_c[:, kt, :],
                        xT_chunks[j][:, kt, :],
                        start=(kt == 0), stop=False,
                    )
                if with_z:
                    z_chunk(j)
                nc.tensor.matmul(
                    yp[:],
                    lBT2_sb[:, ot * 128:(ot + 1) * 128],
                    z_sb[:, j * RC:(j + 1) * RC],
                    start=False, stop=True,
                )
                y_t = y_pool.tile([128, RC], F32, tag="y")
                nc.scalar.activation(y_t[:], yp[:],
                                     mybir.ActivationFunctionType.Identity,
                                     bias=bias_cols[:, ot:ot + 1],
                                     scale=1.0)
                nc.gpsimd.dma_start(
                    out=out_ext[ot * 128:(ot + 1) * 128,
                                j * RC:(j + 1) * RC],
                    in_=y_t[:],
                )

            def column(ot, weffT_c, with_z=False):
                for j in range(NCH):
                    column_chunk(ot, weffT_c, j, with_z=with_z)

            NT = O // 128
            weffs = {0: quant(0)}
            if NT > 1:
                weffs[1] = quant(1)
            x_side()
            for ot in range(NT):
                column(ot, weffs.pop(ot), with_z=(ot == 0))
                if ot + 2 < NT:
                    weffs[ot + 2] = quant(ot + 2)
    nc.compile()
    return nc


def _shard_inputs(x, weight, scale_A, scale_B, bias, lora_A, lora_B,
                  r_groups=R_GROUPS, o_groups=O_GROUPS):
    rows = x.shape[0]
    outs = weight.shape[0]
    Rs, Os = rows // r_groups, outs // o_groups
    lAT = np.ascontiguousarray(lora_A.T)
    in_maps = []
    for c in range(r_groups * o_groups):
        rg, og = divmod(c, o_groups)
        osl = slice(og * Os, (og + 1) * Os)
        in_maps.append({
            "x": np.ascontiguousarray(x[rg * Rs:(rg + 1) * Rs]),
            "w": np.ascontiguousarray(weight[osl]),
            "sAT": np.ascontiguousarray(scale_A[osl].T),
            "sB": np.ascontiguousarray(scale_B),
            "bias": np.ascontiguousarray(bias[osl][None, :]),
            "lAT": lAT,
            "lBT": np.ascontiguousarray(lora_B[osl].T),
        })
    return in_maps


_NC_CACHE = {}


def kernel(x, weight, scale_A, scale_B, bias, lora_A, lora_B, lut,
           _trace=False):
    from concourse.bass_utils import run_bass_kernel_spmd

    x = np.asarray(x, dtype=np.float32)
    weight = np.asarray(weight, dtype=np.float32)
    scale_A = np.asarray(scale_A, dtype=np.float32)
    scale_B = np.asarray(scale_B, dtype=np.float32)
    bias = np.asarray(bias, dtype=np.float32)
    lora_A = np.asarray(lora_A, dtype=np.float32)
    lora_B = np.asarray(lora_B, dtype=np.float32)
    lut = np.asarray(lut, dtype=np.float32)

    B, S, I = x.shape
    OUT = weight.shape[0]
    xf = x.reshape(B * S, I)
    R = (B * S) // R_GROUPS
    O = OUT // O_GROUPS

    d = np.diff(lut.astype(np.float64))
    affine = np.allclose(d, d[0], rtol=0, atol=1e-6 * max(1.0, np.abs(d[0])))
    lut_a = float(lut[0])
    lut_b = float(d.mean())
    nonaffine = None if affine else lut

    key = (R, O, I, lut_a, lut_b, affine)
    if key not in _NC_CACHE:
        _NC_CACHE[key] = build_nc(R, O, I, lut_a, lut_b,
                                  nonaffine_lut=nonaffine)
    nc = _NC_CACHE[key]

    in_maps = _shard_inputs(xf, weight, scale_A, scale_B, bias, lora_A, lora_B)
    res = run_bass_kernel_spmd(nc, in_maps, core_ids=list(range(N_CORES)),
                               trace=_trace)
    y = np.empty((B * S, OUT), np.float32)
    for c in range(N_CORES):
        rg, og = divmod(c, O_GROUPS)
        y[rg * R:(rg + 1) * R, og * O:(og + 1) * O] = \
            res.results[c]["out"].reshape(O, R).T
    out = y.reshape(B, S, OUT)
    if _trace:
        return out, res
    return out


# revision 42
# speedup vs baseline: 1.0815x; 1.0815x over previous
"""Trainium2 Bass kernel for AnemllQATLinear (fake-quant linear + LoRA + bias).

Math (per reference):
    scales = clip(scale_A @ scale_B, 1e-8)              # [OUT, IN], rank-4
    n      = w / scales
    q      = clip(round((n + 1) / step), 0, 15)         # step = 2/15
    w_q    = lut[q] * scales                            # lut affine: lut[q] = a + b*q
    y      = x @ w_q.T + bias + 2.0 * (x @ lora_A.T) @ lora_B.T

Strategy (8 NeuronCores, 4 row-groups x 2 col-groups):
    Each core gets x rows R=2048 and weight rows (out features) O=2048.
    - Fake-quant computed on-chip arithmetically (affine LUT; round via
      the +/-1.5*2^23 magic trick, which is round-half-even like jnp.round).
    - Quantized weight W_eff converted to bf16, transposed via the DMA
      x-bar into [in, out] layout (DRAM bounce), streamed per o-chunk.
    - x cast f32->bf16 during SWDGE DMA, x-bar transposed to xT resident.
    - Main matmul in bf16: psum[r, o] += xT.T @ weffT, with the LoRA
      term (rank-16) and bias (rank-1) accumulated into the same PSUM
      group as extra matmuls.
    - Output written f32, assembled (concat) on host.
"""

import os
import numpy as np

import concourse.bass as bass
import concourse.tile as tile
from concourse import bacc, mybir

F32 = mybir.dt.float32
F32R = mybir.dt.float32r
BF16 = mybir.dt.bfloat16
MAGIC = 12582912.0  # 1.5 * 2**23
LUT_SIZE = 16
STEP_INV = (LUT_SIZE - 1) / 2.0  # 7.5

# full problem shapes
B_FULL, S_FULL, IN_FULL, OUT_FULL = 4, 2048, 4096, 4096
RANK, LORA_R = 4, 16
R_GROUPS, O_GROUPS = 4, 2
N_CORES = 8


def build_nc(R, O, I, lut_a, lut_b, OC=256, IC=512, nonaffine_lut=None):
    """Build the single-core graph (SPMD-launched on all 8 cores).

    R: x rows per core; O: out features per core; I: contraction dim.
    OC: o-chunk for the main matmul (moving free dim). IC: i-chunk for quant.
    """
    KT = I // 128          # number of 128-wide i (contraction) tiles
    RT = R // 128          # r tiles
    ZC = min(512, R)       # z (lora) accumulation chunk of rows
    assert O % OC == 0 and OC % 128 == 0 and I % IC == 0 and IC % 128 == 0

    nc = bacc.Bacc(None, target_bir_lowering=False, debug=False)

    x_in = nc.declare_dram_parameter("x", [R, I], F32, isOutput=False)
    w_in = nc.declare_dram_parameter("w", [O, I], F32, isOutput=False)
    sAT_in = nc.declare_dram_parameter("sAT", [RANK, O], F32, isOutput=False)
    sB_in = nc.declare_dram_parameter("sB", [RANK, I], F32, isOutput=False)
    bias_in = nc.declare_dram_parameter("bias", [1, O], F32, isOutput=False)
    lAT_in = nc.declare_dram_parameter("lAT", [I, LORA_R], F32, isOutput=False)
    lBT_in = nc.declare_dram_parameter("lBT", [LORA_R, O], F32, isOutput=False)
    out_ext = nc.declare_dram_parameter("out", [O, R], F32, isOutput=True)

    with tile.TileContext(nc) as tc:
        with              tc.tile_pool(name="const", bufs=1) as const_pool, \
             tc.tile_pool(name="xside", bufs=3) as x_pool, \
             tc.tile_pool(name="xT", bufs=1) as xT_pool, \
             tc.tile_pool(name="wload", bufs=2) as w_pool, \
             tc.tile_pool(name="qs", bufs=2) as s_pool, \
             tc.tile_pool(name="qchain", bufs=3) as chain_pool, \
             tc.tile_pool(name="qout", bufs=2) as wq_pool, \
             tc.tile_pool(name="weffc", bufs=3) as weff_pool, \
             tc.tile_pool(name="ysb", bufs=1) as y_pool, \
             tc.tile_pool(name="smallio", bufs=2) as small_pool, \
             tc.tile_pool(name="ps_s", bufs=2, space="PSUM") as psum_s, \
             tc.tile_pool(name="ps_y", bufs=4, space="PSUM") as psum_y, \
             tc.tile_pool(name="ps_z", bufs=2, space="PSUM") as psum_z:

            # ---- constants / small inputs ----
            lAT_sb = const_pool.tile([128, KT, LORA_R], BF16)
            nc.gpsimd.dma_start(
                out=lAT_sb[:],
                in_=lAT_in.rearrange("(kt p) r -> p kt r", p=128),
            )
            lBT2_sb = const_pool.tile([LORA_R, O], BF16)
            nc.gpsimd.dma_start(out=lBT2_sb[:], in_=lBT_in[:, :])  # cast f32->bf16
            bias_cols = const_pool.tile([128, O // 128], F32)
            nc.gpsimd.dma_start(
                out=bias_cols[:],
                in_=bias_in.rearrange("1 (ot p) -> p ot", p=128))
            z_sb = const_pool.tile([LORA_R, R], BF16)

            # ---- x side: cast to bf16, x-bar transpose into per-chunk xT
            # tiles (finer tiles -> finer scheduler dependencies), with the
            # LoRA z matmuls interleaved per chunk.
            RC = min(512, R)
            NCH = R // RC
            xT_chunks = [xT_pool.tile([128, KT, RC], BF16, name=f"xT_{j}")
                         for j in range(NCH)]
            XH = 2  # halve transpose count; per-instr overhead dominates

            def x_side():
                for j in range(NCH):
                    for h in range(XH):
                        for rt in range(j * (RC // 128), (j + 1) * (RC // 128)):
                            xbf = x_pool.tile([128, I // XH], BF16, tag="xbf")
                            nc.gpsimd.dma_start(
                                out=xbf[:],
                                in_=x_in[rt * 128:(rt + 1) * 128,
                                         h * (I // XH):(h + 1) * (I // XH)],
                            )
                            lo = (rt % (RC // 128)) * 128
                            nc.sync.dma_start(
                                out=xT_chunks[j][:,
                                                 h * (KT // XH):(h + 1) * (KT // XH),
                                                 lo:lo + 128],
                                in_=xbf[:],
                                transpose=True,
                            )

            def z_chunk(j):
                zp = psum_z.tile([LORA_R, RC], F32, space="PSUM")
                for kt in range(KT):
                    nc.tensor.matmul(
                        zp[:],
                        lAT_sb[:, kt, :],
                        xT_chunks[j][:, kt, :],
                        start=(kt == 0), stop=(kt == KT - 1),
                    )
                # fold the *2.0 LoRA scaling into the psum evacuation
                nc.vector.tensor_scalar(z_sb[:, j * RC:(j + 1) * RC], zp[:],
                                        2.0, None, op0=mybir.AluOpType.mult)

            # ---- main loop: software-pipelined by one o-tile: quant(ot+1) is
            # traced BEFORE the matmuls of ot so every engine queue (PE's
            # scales-mm especially) runs the next tile's quant work while the
            # current column's matmuls stream.

            def quant(ot):
                """Quantize weight o-tile `ot` into a fresh weffT tile."""
                weffT_c = weff_pool.tile([128, KT, 128], BF16, tag="weff",
                                         name=f"weff_{ot}")
                sAT_f = small_pool.tile([RANK, 128], F32, tag="sATf")
                nc.scalar.dma_start(
                    out=sAT_f[:], in_=sAT_in[:, ot * 128:(ot + 1) * 128])
                sAT_t = small_pool.tile([RANK, 128], F32R, tag="sAT")
                nc.vector.tensor_copy(sAT_t[:], sAT_f[:])
                for ic in range(I // IC):
                    w_t = w_pool.tile([128, IC], F32, tag="w")
                    nc.scalar.dma_start(
                        out=w_t[:],
                        in_=w_in[ot * 128:(ot + 1) * 128, ic * IC:(ic + 1) * IC],
                    )
                    sB_f = small_pool.tile([RANK, IC], F32, tag="sBf")
                    nc.scalar.dma_start(
                        out=sB_f[:], in_=sB_in[:, ic * IC:(ic + 1) * IC])
                    sB_t = small_pool.tile([RANK, IC], F32R, tag="sBr")
                    nc.vector.tensor_copy(sB_t[:], sB_f[:])
                    sp = psum_s.tile([128, IC], F32, space="PSUM")
                    nc.tensor.matmul(sp[:], sAT_t[:], sB_t[:],
                                     start=True, stop=True)
                    # s' = max(sp, eps) * step  (so 1/s' = 7.5/s, and
                    # s' is also the final rescale factor: wq=(q+a/b)*s')
                    s_t = s_pool.tile([128, IC], F32, tag="s")
                    nc.vector.tensor_scalar(s_t[:], sp[:], 1e-8, 2.0 / 15.0,
                                            op0=mybir.AluOpType.max,
                                            op1=mybir.AluOpType.mult)
                    r_t = chain_pool.tile([128, IC], F32, tag="chain")
                    nc.vector.reciprocal_approx_fast(r_t[:], s_t[:])
                    n_t = chain_pool.tile([128, IC], F32, tag="chain")
                    nc.vector.tensor_tensor(n_t[:], w_t[:], r_t[:],
                                            op=mybir.AluOpType.mult)
                    t_t = chain_pool.tile([128, IC], F32, tag="chain")
                    nc.vector.tensor_scalar(t_t[:], n_t[:], STEP_INV, MAGIC,
                                            op0=mybir.AluOpType.add,
                                            op1=mybir.AluOpType.add)
                    q_t = chain_pool.tile([128, IC], F32, tag="chain")
                    nc.vector.tensor_scalar(q_t[:], t_t[:], MAGIC,
                                            float(LUT_SIZE - 1),
                                            op0=mybir.AluOpType.subtract,
                                            op1=mybir.AluOpType.min)
                    u_t = chain_pool.tile([128, IC], F32, tag="chain")
                    if nonaffine_lut is None:
                        nc.vector.tensor_scalar(u_t[:], q_t[:], 0.0,
                                                float(lut_a / lut_b),
                                                op0=mybir.AluOpType.max,
                                                op1=mybir.AluOpType.add)
                    else:
                        # generic LUT: u = (lut[0] + sum_k d_k*(q >= k-0.5))/b
                        lut = nonaffine_lut
                        b_scl = 2.0 / 15.0
                        acc = chain_pool.tile([128, IC], F32, tag="nacc")
                        nc.vector.tensor_scalar(acc[:], q_t[:], 0.0,
                                                float(lut[0]) / b_scl,
                                                op0=mybir.AluOpType.mult,
                                                op1=mybir.AluOpType.add)
                        for k in range(1, LUT_SIZE):
                            d_k = float(lut[k] - lut[k - 1])
                            ind = chain_pool.tile([128, IC], F32, tag="nind")
                            nc.vector.tensor_scalar(ind[:], q_t[:], k - 0.5,
                                                    d_k / b_scl,
                                                    op0=mybir.AluOpType.is_ge,
                                                    op1=mybir.AluOpType.mult)
                            acc2 = chain_pool.tile([128, IC], F32, tag="nacc")
                            nc.vector.tensor_tensor(acc2[:], acc[:], ind[:],
                                                    op=mybir.AluOpType.add)
                            acc = acc2
                        u_t = acc
                    if nonaffine_lut is not None or abs(lut_b - 2.0 / 15.0) < 1e-12:
                        s_fin = s_t
                    else:
                        s_fin = s_pool.tile([128, IC], F32, tag="s2")
                        nc.vector.tensor_scalar(s_fin[:], s_t[:],
                                                float(lut_b / (2.0 / 15.0)),
                                                None, op0=mybir.AluOpType.mult)
                    # batch four ic-chunks into one wq tile -> one transpose
                    if ic % 4 == 0:
                        wq_t = wq_pool.tile([128, 4 * IC], BF16, tag="wq")
                    part = (ic % 4) * IC
                    nc.vector.tensor_tensor(wq_t[:, part:part + IC], u_t[:],
                                            s_fin[:], op=mybir.AluOpType.mult)
                    if ic % 4 == 3:
                        nc.sync.dma_start(
                            out=weffT_c[:, (ic - 3) * (IC // 128):
                                        (ic + 1) * (IC // 128), :],
                            in_=wq_t[:],
                            transpose=True,
                        )
                return weffT_c

            def column_chunk(ot, weffT_c, j, with_z=False):
                """One r-chunk of yT[o_tile] = W_eff[o_tile] @ x.T + lora;
                bias folds into the ACT psum evacuation."""
                yp = psum_y.tile([128, RC], F32, space="PSUM",
                                 tag="yp", name=f"yp{j}_{ot}")
                for kt in range(KT):
                    nc.tensor.matmul(
                        yp[:],
                        weffT_c[:, kt, :],
                        xT_chunks[j][:, kt, :],
                        start=(kt == 0), stop=False,
                    )
                if with_z:
                    z_chunk(j)
                nc.tensor.matmul(
                    yp[:],
                    lBT2_sb[:, ot * 128:(ot + 1) * 128],
                    z_sb[:, j * RC:(j + 1) * RC],
                    start=False, stop=True,
                )
                y_t = y_pool.tile([128, RC], F32, tag="y")
                nc.scalar.activation(y_t[:], yp[:],
                                     mybir.ActivationFunctionType.Identity,
                                     bias=bias_cols[:, ot:ot + 1],
                                     scale=1.0)
                nc.gpsimd.dma_start(
                    out=out_ext[ot * 128:(ot + 1) * 128,
                                j * RC:(j + 1) * RC],
                    in_=y_t[:],
                )

            def column(ot, weffT_c, with_z=False):
                for j in range(NCH):
                    column_chunk(ot, weffT_c, j, with_z=with_z)

            NT = O // 128
            weffs = {0: quant(0)}
            if NT > 1:
                weffs[1] = quant(1)
            x_side()
            for ot in range(NT):
                column(ot, weffs.pop(ot), with_z=(ot == 0))
                if ot + 2 < NT:
                    weffs[ot + 2] = quant(ot + 2)
    nc.compile()
    return nc


def _shard_inputs(x, weight, scale_A, scale_B, bias, lora_A, lora_B,
                  r_groups=R_GROUPS, o_groups=O_GROUPS):
    rows = x.shape[0]
    outs = weight.shape[0]
    Rs, Os = rows // r_groups, outs // o_groups
    lAT = np.ascontiguousarray(lora_A.T)
    in_maps = []
    for c in range(r_groups * o_groups):
        rg, og = divmod(c, o_groups)
        osl = slice(og * Os, (og + 1) * Os)
        in_maps.append({
            "x": np.ascontiguousarray(x[rg * Rs:(rg + 1) * Rs]),
            "w": np.ascontiguousarray(weight[osl]),
            "sAT": np.ascontiguousarray(scale_A[osl].T),
            "sB": np.ascontiguousarray(scale_B),
            "bias": np.ascontiguousarray(bias[osl][None, :]),
            "lAT": lAT,
            "lBT": np.ascontiguousarray(lora_B[osl].T),
        })
    return in_maps


_NC_CACHE = {}


def kernel(x, weight, scale_A, scale_B, bias, lora_A, lora_B, lut,
           _trace=False):
    from concourse.bass_utils import run_bass_kernel_spmd

    x = np.asarray(x, dtype=np.float32)
    weight = np.asarray(weight, dtype=np.float32)
    scale_A = np.asarray(scale_A, dtype=np.float32)
    scale_B = np.asarray(scale_B, dtype=np.float32)
    bias = np.asarray(bias, dtype=np.float32)
    lora_A = np.asarray(lora_A, dtype=np.float32)
    lora_B = np.asarray(lora_B, dtype=np.float32)
    lut = np.asarray(lut, dtype=np.float32)

    B, S, I = x.shape
    OUT = weight.shape[0]
    xf = x.reshape(B * S, I)
    R = (B * S) // R_GROUPS
    O = OUT // O_GROUPS

    d = np.diff(lut.astype(np.float64))
    affine = np.allclose(d, d[0], rtol=0, atol=1e-6 * max(1.0, np.abs(d[0])))
    lut_a = float(lut[0])
    lut_b = float(d.mean())
    nonaffine = None if affine else lut

    key = (R, O, I, lut_a, lut_b, affine)
    if key not in _NC_CACHE:
        _NC_CACHE[key] = build_nc(R, O, I, lut_a, lut_b,
                                  nonaffine_lut=nonaffine)
    nc = _NC_CACHE[key]

    in_maps = _shard_inputs(xf, weight, scale_A, scale_B, bias, lora_A, lora_B)
    res = run_bass_kernel_spmd(nc, in_maps, core_ids=list(range(N_CORES)),
                               trace=_trace)
    y = np.empty((B * S, OUT), np.float32)
    for c in range(N_CORES):
        rg, og = divmod(c, O_GROUPS)
        y[rg * R:(rg + 1) * R, og * O:(og + 1) * O] = \
            res.results[c]["out"].reshape(O, R).T
    out = y.reshape(B, S, OUT)
    if _trace:
        return out, res
    return out
